# revision 1
# baseline (speedup 1.0000x reference)
"""CTC loss (nn.CTCLoss, blank=0, reduction='mean', zero_infinity=True) for
T=160, B=64, C=6625, S=25 on 8 TRN2 NeuronCores.

Sharding: data-parallel over batch — 8 of the 64 samples per core.

Algorithm (mathematically identical to the log-domain reference): the CTC
forward DP runs in the probability domain with periodic rescaling.  With
p[t,s] = exp(score of extended-target symbol s at time t) and
q = p * skip_mask, each step is

    alpha_new[s] = alpha[s-2]*q[t,s] + alpha[s-1]*p[t,s] + alpha[s]*p[t,s]

computed as TWO Vector-engine ops on an [8, 51, 3] tile: one elementwise
multiply of the overlapped 3-tap view of alpha against a pre-interleaved
(q,p,p) coefficient array, then a strided reduce_sum over the tap axis.
Every 8 steps the per-sample sum is folded out as log(scale).  Only the 51
extended-target class columns are gathered from the predictions shard
(indirect DMA); the other 6574 classes are never read.
"""

import numpy as np

import concourse.bacc as bacc
import concourse.bass as bass
import concourse.mybir as mybir
import concourse.tile as tile
from concourse.bass_utils import run_bass_kernel_spmd

T = 160
B = 64
C = 6625
S = 25
L = 2 * S + 1  # 51
NCORES = 8
BLOC = B // NCORES  # 8 samples per core
NORM_EVERY = 8
NG = (BLOC * L + 127) // 128  # 4 gather blocks of 128 rows (408 pad to 512)

F32 = mybir.dt.float32
I32 = mybir.dt.int32
ALU = mybir.AluOpType
ACTF = mybir.ActivationFunctionType
AXIS = mybir.AxisListType


def _combined_act_tables(module_arch):
    """Force Exp and Ln into one table set (one ~2.7us ACT_TABLE_LOAD instead
    of two).  Set names/positions are preserved (ids are positional); sets
    other than the combined exp+ln one just stop advertising Exp/Ln."""
    tables = dict(_orig_get_activation_tables(module_arch))
    both = {ACTF.Exp, ACTF.Ln}
    combined = [n for n, fns in tables.items() if both <= fns]
    if combined:
        keep = combined[0]
        for n in list(tables):
            if n != keep and (tables[n] & both):
                tables[n] = tables[n] - both
    return tables


_orig_get_activation_tables = bacc.get_activation_tables


def build_nc(loop_T: int = T) -> bass.Bass:
    bacc.get_activation_tables = _combined_act_tables
    nc = bacc.Bacc("TRN2", target_bir_lowering=False)

    preds = nc.dram_tensor("preds", [BLOC * C, T], F32, kind="ExternalInput")
    gidx_d = nc.dram_tensor("gidx", [128, NG], I32, kind="ExternalInput")
    maskc_d = nc.dram_tensor("maskcol", [128, NG], F32, kind="ExternalInput")
    oneh_d = nc.dram_tensor("onehot", [BLOC, L], F32, kind="ExternalInput")
    out_d = nc.dram_tensor("out2", [BLOC, 2], F32, kind="ExternalOutput")
    pscr_p = nc.dram_tensor("pscr_p", [128 * NG, T], F32)  # internal bounce
    pscr_q = nc.dram_tensor("pscr_q", [128 * NG, T], F32)

    n_scales = (T - 2) // NORM_EVERY  # t=7,15,...,151 -> 19 rescales
    with tile.TileContext(nc) as tc:
        with (
            tc.tile_pool(name="big", bufs=1) as bigp,
            tc.tile_pool(name="small", bufs=1) as smallp,
            tc.tile_pool(name="tmp", bufs=2) as tmpp,
        ):
            G = bigp.tile([128, NG, T], F32, tag="G")
            Gp = bigp.tile([128, NG, T], F32, tag="Gp")
            Gq = bigp.tile([128, NG, T], F32, tag="Gq")
            # PPQ[b, l, j, t] = (q, p, p)[j] at (b, l, t); chunked along t so
            # the loop can start as soon as the first chunk's DMAs land.
            TCH = 40
            NCH = (T + TCH - 1) // TCH
            PPQs = [
                bigp.tile([BLOC, L, 3, TCH], F32, tag=f"PPQ{c}", name=f"PPQ{c}")
                for c in range(NCH)
            ]

            gidx = smallp.tile([128, NG], I32, tag="gidx")
            maskc = smallp.tile([128, NG], F32, tag="maskc")
            oneh = smallp.tile([BLOC, L], F32, tag="oneh")
            X = smallp.tile([BLOC, L + 2], F32, tag="X")
            Y = smallp.tile([BLOC, L + 2], F32, tag="Y")
            scales = smallp.tile([BLOC, n_scales + 1], F32, tag="scales")
            logs = smallp.tile([BLOC, n_scales + 1], F32, tag="logs")
            rcol = smallp.tile([BLOC, 1], F32, tag="rcol")
            out_s = smallp.tile([BLOC, 2], F32, tag="out_s")

            nc.sync.dma_start(out=gidx[:, :], in_=gidx_d[:, :])
            nc.sync.dma_start(out=maskc[:, :], in_=maskc_d[:, :])
            nc.sync.dma_start(out=oneh[:, :], in_=oneh_d[:, :])

            # Gather row-per-partition: G[p, j, :] = preds[gidx[p, j], :]
            for j in range(NG):
                nc.gpsimd.indirect_dma_start(
                    out=G[:, j, :],
                    out_offset=None,
                    in_=preds[:, :],
                    in_offset=bass.IndirectOffsetOnAxis(ap=gidx[:, j : j + 1], axis=0),
                )
            # Pipeline exp/mask/bounce per t-chunk so the DP loop can start
            # once chunk 0 lands; chunks 1..3 overlap with the loop.
            # pscr rows are r = j*128 + p  (flat b-major row id b*L + l).
            for c in range(NCH):
                cs = slice(c * TCH, (c + 1) * TCH)
                nc.scalar.activation(Gp[:, :, cs], G[:, :, cs], ACTF.Exp)
                for j in range(NG):
                    # per-partition scalar multiply on the (otherwise idle)
                    # ACT engine, keeping the Vector engine free for the loop
                    nc.scalar.mul(
                        Gq[:, j, cs], Gp[:, j, cs], maskc[:, j : j + 1]
                    )
                out_ap_p = bass.AP(
                    pscr_p, c * TCH, [[T, 128], [128 * T, NG], [1, TCH]]
                )
                out_ap_q = bass.AP(
                    pscr_q, c * TCH, [[T, 128], [128 * T, NG], [1, TCH]]
                )
                nc.sync.dma_start(out=out_ap_p, in_=Gp[:, :, cs])
                nc.sync.dma_start(out=out_ap_q, in_=Gq[:, :, cs])
                in_p = bass.AP(pscr_p, c * TCH, [[L * T, BLOC], [T, L], [1, TCH]])
                in_q = bass.AP(pscr_q, c * TCH, [[L * T, BLOC], [T, L], [1, TCH]])
                nc.sync.dma_start(out=PPQs[c][:, :, 0, :], in_=in_q)
                nc.sync.dma_start(out=PPQs[c][:, :, 1, :], in_=in_p)
                nc.sync.dma_start(out=PPQs[c][:, :, 2, :], in_=in_p)

            # alpha0: [p(0,0), p(0,1), 0, ...] in padded cols 2:4 of X
            nc.vector.memset(X[:, :], 0.0)
            nc.vector.memset(Y[:, :], 0.0)
            nc.vector.tensor_copy(X[:, 2:4], PPQs[0][:, 0:2, 1, 0])

            cur, nxt = X, Y
            apply_norm = False
            for t in range(1, loop_T):
                ppq_t = PPQs[t // TCH][:, :, :, t % TCH]
                xap = cur[:, :]
                xxx = bass.AP(xap.tensor, xap.offset, [xap.ap[0], [1, L], [1, 3]])

                M = tmpp.tile([BLOC, L, 3], F32, tag="M")
                is_norm = t % NORM_EVERY == NORM_EVERY - 1 and t < T - 1
                k = t // NORM_EVERY
                if apply_norm or is_norm:
                    # stt form: optional rescale via scalar, and on norm steps
                    # the accum_out gives sum(M) = sum(alpha_new) for free.
                    # (tensor_tensor_reduce would fuse this cheaper per the
                    # cost model but fails on HW with these overlapped APs.)
                    nc.vector.scalar_tensor_tensor(
                        out=M[:, :, :], in0=xxx,
                        scalar=rcol[:, :] if apply_norm else 1.0, in1=ppq_t,
                        op0=ALU.mult, op1=ALU.mult,
                        accum_out=scales[:, k : k + 1] if is_norm else None,
                    )
                    apply_norm = False
                else:
                    nc.vector.tensor_tensor(
                        out=M[:, :, :], in0=xxx, in1=ppq_t, op=ALU.mult
                    )
                nc.vector.tensor_reduce(
                    out=nxt[:, 2 : L + 2], in_=M[:, :, :], axis=AXIS.X, op=ALU.add
                )
                if is_norm:
                    nc.vector.reciprocal(out=rcol[:, :], in_=scales[:, k : k + 1])
                    apply_norm = True
                cur, nxt = nxt, cur

            # Final-state sum (one more rescale so dot is well-conditioned),
            # then dot = sum_s (alpha[s]/s_fin) * onehot[s].  log(dot) happens
            # on the host: the ACT Ln table clamps inputs below ~1e-20 and dot
            # can be that small; the device only ever Ln's the window sums,
            # which are safely in range.
            nc.vector.tensor_reduce(
                out=scales[:, n_scales : n_scales + 1], in_=cur[:, 2 : L + 2],
                axis=AXIS.X, op=ALU.add,
            )
            nc.vector.reciprocal(out=rcol[:, :], in_=scales[:, n_scales : n_scales + 1])
            z2 = tmpp.tile([BLOC, L], F32, tag="z2")
            nc.vector.scalar_tensor_tensor(
                out=z2[:, :], in0=cur[:, 2 : L + 2], scalar=rcol[:, :], in1=oneh[:, :],
                op0=ALU.mult, op1=ALU.mult,
                accum_out=out_s[:, 1:2],
            )
            # out_s[:, 0] = sum_k log(scale_k) + log(s_fin)
            nc.scalar.activation(logs[:, :], scales[:, :], ACTF.Ln)
            nc.vector.tensor_reduce(
                out=out_s[:, 0:1], in_=logs[:, :], axis=AXIS.X, op=ALU.add
            )
            nc.sync.dma_start(out=out_d[:, :], in_=out_s[:, :])

    try:
        nc.finalize()
    finally:
        bacc.get_activation_tables = _orig_get_activation_tables
    return nc


def host_prep(predictions, targets, target_lengths):
    """Host-side shard + index prep. Returns per-core input maps."""
    predictions = np.asarray(predictions, dtype=np.float32)
    targets = np.asarray(targets)
    target_lengths = np.asarray(target_lengths)

    ext = np.zeros((B, L), dtype=np.int64)
    ext[:, 1::2] = targets
    mask01 = np.zeros((B, L), dtype=np.float32)
    mask01[:, 3::2] = (targets[:, 1:] != targets[:, :-1]).astype(np.float32)
    onehot = np.zeros((B, L), dtype=np.float32)
    idx = (2 * target_lengths).astype(np.int64)
    onehot[np.arange(B), idx] = 1.0
    onehot[np.arange(B), idx - 1] = 1.0

    in_maps = []
    for k in range(NCORES):
        bsl = slice(k * BLOC, (k + 1) * BLOC)
        # [T, BLOC, C] -> [BLOC, C, T] contiguous -> flat [BLOC*C, T]
        pshard = np.ascontiguousarray(
            predictions[:, bsl, :].transpose(1, 2, 0)
        ).reshape(BLOC * C, T)
        gidx = (
            np.arange(BLOC, dtype=np.int64)[:, None] * C + ext[bsl]
        ).astype(np.int32)
        # b-major flat row list, padded to 128*NG, as [128, NG] column-blocks
        gflat = np.zeros(128 * NG, dtype=np.int32)
        gflat[: BLOC * L] = gidx.reshape(-1)
        mflat = np.zeros(128 * NG, dtype=np.float32)
        mflat[: BLOC * L] = mask01[bsl].reshape(-1)
        in_maps.append(
            {
                "preds": pshard,
                "gidx": gflat.reshape(NG, 128).T.copy(),
                "maskcol": mflat.reshape(NG, 128).T.copy(),
                "onehot": onehot[bsl],
            }
        )
    return in_maps


_NC_CACHE = {}


def kernel(predictions, targets, target_lengths):
    if "nc" not in _NC_CACHE:
        _NC_CACHE["nc"] = build_nc()
    nc = _NC_CACHE["nc"]

    in_maps = host_prep(predictions, targets, target_lengths)
    res = run_bass_kernel_spmd(nc, in_maps, core_ids=list(range(NCORES)))
    return finish(res.results, target_lengths)


def finish(results, target_lengths):
    out2 = np.concatenate([r["out2"].reshape(BLOC, 2) for r in results])
    slogsum, dot = out2[:, 0], out2[:, 1]
    with np.errstate(divide="ignore"):
        nll = -(np.log(dot.astype(np.float32)).astype(np.float32) + slogsum)
    lengths = np.asarray(target_lengths).astype(np.float32)
    per = np.where(nll >= 1e29, np.float32(0.0), nll / lengths)
    return np.array(per.mean(), dtype=np.float32)



# revision 8
# speedup vs baseline: 1.4125x; 1.4125x over previous
"""CTC loss (nn.CTCLoss, blank=0, reduction='mean', zero_infinity=True) for
T=160, B=64, C=6625, S=25 on 8 TRN2 NeuronCores.

Sharding: data-parallel over batch - 8 of the 64 samples per core.

Algorithm (mathematically identical to the log-domain reference): the CTC
forward DP runs in the probability domain with periodic rescaling.  With
p[t,s] = exp(score of extended-target symbol s at time t) and
q[t,s] = p[t,s] where the skip transition s-2 -> s is allowed else 0,
each step is

    alpha_new[s] = alpha[s-2]*q[t,s] + alpha[s-1]*p[t,s] + alpha[s]*p[t,s]

computed as TWO Vector-engine ops on an [8, 51, 3] tile: an elementwise
multiply of the overlapped 3-tap view of alpha against the pre-interleaved
(q,p,p) coefficient tile, then a strided reduce_sum over the tap axis.
Every 8 steps the per-sample sum is folded out as a raw scale; the final
alpha row and the raw scales go back to the host, which finishes with
log()s (tiny, numerically safe there).

Key device-side structure:
  - only the 51 extended-target class rows per sample are gathered from
    the predictions shard (indirect DMA); q is NOT gathered separately -
    it is exp(score + lmask) where lmask is 0 / -100 per (sample,state),
    applied as the per-partition bias of the Exp activation.
  - the (q,p,p) tap tile is built by SBUF->SBUF DMAs straight out of the
    exp'd gather tile (no DRAM bounce).
  - same-engine (DVE->DVE) scheduler dependencies are demoted from
    semaphore-synced to program-order-only before wait assignment: the DVE
    drains its pipeline between ops, so back-to-back dependent ops on one
    engine need no semaphore round-trip.
"""

import os

import numpy as np

import bass_rust as _bass_rust
import concourse.bacc as bacc
import concourse.bass as bass
import concourse.mybir as mybir
import concourse.tile as tile
from concourse.bass_utils import run_bass_kernel_spmd

T = 160
B = 64
C = 6625
S = 25
L = 2 * S + 1  # 51
NCORES = 8
BLOC = B // NCORES  # 8 samples per core
NORM_EVERY = 8
NSC = (T - 2) // NORM_EVERY  # 19 rescales at t=7,15,...,151
NG = 4  # gather blocks: 2 samples per 128-partition block
TCH = 40
NCH = T // TCH
LMASK_OFF = -100.0  # exp(score - 100) == 0 for all practical purposes

F32 = mybir.dt.float32
I32 = mybir.dt.int32
ALU = mybir.AluOpType
ACTF = mybir.ActivationFunctionType
AXIS = mybir.AxisListType

# Norm cadence: scale captured at t = 8k-1, applied (with 2 steps of slack
# so nothing ever blocks) at t = 8k+1.
APPLY_DELAY = 2


def _demote_same_engine_syncs(ordered):
    """Demote DVE->DVE sync dependencies to nosync (program order only).

    The Tile scheduler conservatively semaphore-syncs every data
    dependency, including between consecutive instructions on the same
    engine.  Engine execution is in-order and the DVE flushes its 8-slice
    pipe between ops, so same-engine hazards are already safe; the
    semaphore round-trip only adds ~95ns of propagation per op on the
    serial DP chain.  DMA copies complete asynchronously and keep their
    syncs, as does everything cross-engine.
    """
    info = {}
    for insts in ordered.values():
        for i in insts:
            info[i.name] = (i.engine, i.opcode)
    for insts in ordered.values():
        for i in insts:
            if i.engine != mybir.EngineType.DVE or i.opcode == "DMACopy":
                continue
            syncs = list(i.sync_dependency_names())
            demote = [
                d
                for d in syncs
                if info.get(d, (None, None))[0] == mybir.EngineType.DVE
                and info[d][1] != "DMACopy"
            ]
            if not demote:
                continue
            keep = [d for d in syncs if d not in set(demote)]
            cur_ns = list(i.nosync_dependency_names())
            i.take_sync_dependencies()
            i.take_nosync_dependencies()
            i.set_sync_dependencies(_bass_rust.InstructionNameOrderedSet(keep))
            i.set_nosync_dependencies(
                _bass_rust.InstructionNameOrderedSet(
                    list(dict.fromkeys(cur_ns + demote))
                )
            )


_ORIG_TCW = tile.TileClockWait


def _demoting_tcw(tc, ordered):
    _demote_same_engine_syncs(ordered)
    return _ORIG_TCW(tc, ordered)


def build_nc(loop_T: int = T) -> bass.Bass:
    if os.environ.get("CTC_NO_DEMOTE", "0") != "1":
        tile.TileClockWait = _demoting_tcw
    try:
        nc = bacc.Bacc("TRN2", target_bir_lowering=False)

        preds = nc.dram_tensor("preds", [BLOC * C, T], F32, kind="ExternalInput")
        gidx_d = nc.dram_tensor("gidx", [128, NG], I32, kind="ExternalInput")
        lmask_d = nc.dram_tensor("lmask", [128, NG], F32, kind="ExternalInput")
        out_d = nc.dram_tensor("out2", [BLOC, L + NSC], F32, kind="ExternalOutput")

        with tile.TileContext(nc) as tc:
            with (
                tc.tile_pool(name="big", bufs=1) as bigp,
                tc.tile_pool(name="small", bufs=1) as smallp,
                tc.tile_pool(name="tmp", bufs=2) as tmpp,
            ):
                G = bigp.tile([128, NG, T], F32, tag="G")
                Gp = bigp.tile([128, NG, T], F32, tag="Gp")
                Gq = bigp.tile([128, NG, T], F32, tag="Gq")
                # state dim padded 51->64 so (b%2, l) merges into one
                # 128-count DMA dim (AP balancer allows max 3 dims)
                PPQ = bigp.tile([BLOC, 64, 3, T], F32, tag="PPQ")

                gidx = smallp.tile([128, NG], I32, tag="gidx")
                lmask = smallp.tile([128, NG], F32, tag="lmask")
                X = smallp.tile([BLOC, L + 2], F32, tag="X")
                Y = smallp.tile([BLOC, L + 2], F32, tag="Y")
                scales = smallp.tile([BLOC, NSC], F32, tag="scales")
                rcol = smallp.tile([BLOC, 2], F32, tag="rcol")
                ones = smallp.tile([BLOC, 1], F32, tag="ones")
                rtmp = smallp.tile([BLOC, 2], F32, tag="rtmp")
                dummy = smallp.tile([BLOC, 1], F32, tag="dummy")

                # Trigger the Exp act-table load immediately (it costs
                # 1.3us and otherwise lands on the first-chunk critical
                # path, after the gather).
                nc.vector.memset(dummy[:, :], 0.0)
                nc.scalar.activation(dummy[:, :], dummy[:, :], ACTF.Exp)
                nc.gpsimd.memset(ones[:, :], 1.0)

                nc.sync.dma_start(out=gidx[:, :], in_=gidx_d[:, :])
                nc.sync.dma_start(out=lmask[:, :], in_=lmask_d[:, :])

                # Gather row-per-partition: G[p, j, :] = preds[gidx[p, j], :]
                for j in range(NG):
                    nc.gpsimd.indirect_dma_start(
                        out=G[:, j, :],
                        out_offset=None,
                        in_=preds[:, :],
                        in_offset=bass.IndirectOffsetOnAxis(
                            ap=gidx[:, j : j + 1], axis=0
                        ),
                    )

                # Per t-chunk: exp (+ lmask bias for the q side), then build
                # the (q,p,p) tap tile with SBUF->SBUF DMAs.  Gather row
                # layout: partition p = b*16 + l//4, block j = l%4 - chosen
                # so both DMA sides reduce to legal partition-major 3-dim
                # APs (the l%4 part rides in the within-row block offset).
                W_G = NG * T
                W_PPQ = 64 * 3 * T
                gp_ap = Gp[:, :, :]
                gq_ap = Gq[:, :, :]
                ppq_ap = PPQ[:, :, :, :]
                for c in range(NCH):
                    cs = slice(c * TCH, (c + 1) * TCH)
                    nc.scalar.activation(Gp[:, :, cs], G[:, :, cs], ACTF.Exp)
                    for j in range(NG):
                        nc.scalar.activation(
                            Gq[:, j, cs],
                            G[:, j, cs],
                            ACTF.Exp,
                            bias=lmask[:, j : j + 1],
                        )
                    # iteration dims ((b,l//4):128, l%4:4, t:TCH)
                    in_dims = [[W_G, 128], [T, NG], [1, TCH]]
                    out_dims = [[W_PPQ, BLOC], [3 * T, 64], [1, TCH]]
                    nc.scalar.dma_start(
                        out=bass.AP(
                            ppq_ap.tensor, ppq_ap.offset + 0 * T + c * TCH, out_dims
                        ),
                        in_=bass.AP(gq_ap.tensor, gq_ap.offset + c * TCH, in_dims),
                    )
                    nc.sync.dma_start(
                        out=bass.AP(
                            ppq_ap.tensor, ppq_ap.offset + 1 * T + c * TCH, out_dims
                        ),
                        in_=bass.AP(gp_ap.tensor, gp_ap.offset + c * TCH, in_dims),
                    )
                    nc.sync.dma_start(
                        out=bass.AP(
                            ppq_ap.tensor, ppq_ap.offset + 2 * T + c * TCH, out_dims
                        ),
                        in_=bass.AP(gp_ap.tensor, gp_ap.offset + c * TCH, in_dims),
                    )

                # alpha0: [p(0,0), p(0,1), 0, ...] in padded cols 2:4 of X
                nc.vector.memset(X[:, :], 0.0)
                nc.vector.memset(Y[:, :], 0.0)
                nc.vector.tensor_copy(X[:, 2:4], PPQ[:, 0:2, 1, 0])

                cur, nxt = X, Y
                apply_at = {}  # step t -> scale index to divide out
                for t in range(1, loop_T):
                    ppq_t = PPQ[:, 0:L, :, t]
                    xap = cur[:, :]
                    xxx = bass.AP(
                        xap.tensor, xap.offset, [xap.ap[0], [1, L], [1, 3]]
                    )

                    M = tmpp.tile([BLOC, L, 3], F32, tag="M")
                    is_norm = t % NORM_EVERY == NORM_EVERY - 1 and t < T - 1
                    k = t // NORM_EVERY
                    ak = apply_at.pop(t, None)
                    if ak is not None or is_norm:
                        nc.vector.scalar_tensor_tensor(
                            out=M[:, :, :],
                            in0=xxx,
                            scalar=rcol[:, ak % 2 : ak % 2 + 1]
                            if ak is not None
                            else 1.0,
                            in1=ppq_t,
                            op0=ALU.mult,
                            op1=ALU.mult,
                            accum_out=scales[:, k : k + 1] if is_norm else None,
                        )
                    else:
                        nc.vector.tensor_tensor(
                            out=M[:, :, :], in0=xxx, in1=ppq_t, op=ALU.mult
                        )
                    nc.vector.tensor_reduce(
                        out=nxt[:, 2 : L + 2],
                        in_=M[:, :, :],
                        axis=AXIS.X,
                        op=ALU.add,
                    )
                    if is_norm:
                        # reciprocal off the DVE chain: gpsimd has 2 steps
                        # of slack before the result is consumed.  scales
                        # must stay raw for the host, so recip a copy.
                        nc.gpsimd.tensor_copy(
                            rtmp[:, k % 2 : k % 2 + 1], scales[:, k : k + 1]
                        )
                        nc.gpsimd.normalize_recip(
                            out_ap=rcol[:, k % 2 : k % 2 + 1],
                            in_ap=ones[:, :],
                            denom_ap=rtmp[:, k % 2 : k % 2 + 1],
                        )
                        apply_at[t + APPLY_DELAY] = k
                    cur, nxt = nxt, cur

                # Ship the raw final alpha and the raw window sums; the
                # host finishes with logs (safe range there).
                nc.sync.dma_start(out=out_d[:, 0:L], in_=cur[:, 2 : L + 2])
                nc.scalar.dma_start(out=out_d[:, L : L + NSC], in_=scales[:, :])

        nc.finalize()
    finally:
        tile.TileClockWait = _ORIG_TCW
    return nc


def host_prep(predictions, targets, target_lengths):
    """Host-side shard + index prep. Returns per-core input maps."""
    predictions = np.asarray(predictions, dtype=np.float32)
    targets = np.asarray(targets)
    target_lengths = np.asarray(target_lengths)

    ext = np.zeros((B, L), dtype=np.int64)
    ext[:, 1::2] = targets
    mask01 = np.zeros((B, L), dtype=np.float32)
    mask01[:, 3::2] = (targets[:, 1:] != targets[:, :-1]).astype(np.float32)

    in_maps = []
    for kk in range(NCORES):
        bsl = slice(kk * BLOC, (kk + 1) * BLOC)
        # [T, BLOC, C] -> [BLOC, C, T] contiguous -> flat [BLOC*C, T]
        pshard = np.ascontiguousarray(
            predictions[:, bsl, :].transpose(1, 2, 0)
        ).reshape(BLOC * C, T)
        gidx = np.zeros((128, NG), dtype=np.int32)
        lm = np.full((128, NG), LMASK_OFF, dtype=np.float32)
        for b in range(BLOC):
            for l in range(L):
                p, j = b * 16 + l // 4, l % 4
                gidx[p, j] = b * C + ext[kk * BLOC + b, l]
                if mask01[kk * BLOC + b, l] > 0:
                    lm[p, j] = 0.0
        in_maps.append({"preds": pshard, "gidx": gidx, "lmask": lm})
    return in_maps


_NC_CACHE = {}


def kernel(predictions, targets, target_lengths):
    if "nc" not in _NC_CACHE:
        _NC_CACHE["nc"] = build_nc()
    nc = _NC_CACHE["nc"]

    in_maps = host_prep(predictions, targets, target_lengths)
    res = run_bass_kernel_spmd(nc, in_maps, core_ids=list(range(NCORES)))
    return finish(res.results, target_lengths)


def finish(results, target_lengths):
    out = np.concatenate(
        [np.asarray(r["out2"]).reshape(BLOC, L + NSC) for r in results]
    )
    alpha, scales = out[:, :L].astype(np.float64), out[:, L:].astype(np.float64)
    lengths = np.asarray(target_lengths)
    idx = (2 * lengths).astype(np.int64)
    val = alpha[np.arange(B), idx] + alpha[np.arange(B), idx - 1]
    with np.errstate(divide="ignore", invalid="ignore"):
        nll = -(np.log(val) + np.log(scales).sum(axis=1))
    bad = ~np.isfinite(nll) | (nll >= 1e29)
    per = np.where(bad, 0.0, nll / lengths.astype(np.float64))
    return np.float32(per.mean())


# revision 18
# speedup vs baseline: 1.6158x; 1.1439x over previous
"""CTC loss (nn.CTCLoss, blank=0, reduction='mean', zero_infinity=True) for
T=160, B=64, C=6625, S=25 on 8 TRN2 NeuronCores.

Sharding: data-parallel over batch - 8 of the 64 samples per core.

Algorithm (mathematically identical to the log-domain reference): the CTC
forward DP runs in the probability domain with periodic rescaling.  With
p[t,s] = exp(score of extended-target symbol s at time t) and
q[t,s] = p[t,s] where the skip transition s-2 -> s is allowed else 0,
one step is

    alpha'[s] = alpha[s-2]*q[t,s] + alpha[s-1]*p[t,s] + alpha[s]*p[t,s]

i.e. alpha' = M_t alpha with M_t banded (3 bands).  Each DP step costs two
serial Vector-engine ops (elementwise multiply of an overlapped tap view
of alpha, then a strided reduce over the tap axis), and this serial DVE
chain is the kernel's critical path.  So for t >= K1 TWO steps are fused:
N = M_{t+1} M_t is a 5-band matrix whose bands are precomputed on the
otherwise-idle GPSIMD engine, halving the number of serial DVE ops for
those steps (one 5-tap multiply + reduce per fused pair).

Other structure:
  - only the 51 extended-target class rows per sample are gathered from
    the predictions shard (one multi-row indirect DMA); q is not gathered:
    it is exp(score + lmask), lmask in {0, -100}, applied as the
    per-partition bias of the Exp activation.
  - band precompute needs state-shifted (s-1, s-2) copies of p and q;
    states live across partitions in the gather layout, so the shifts are
    materialized by SBUF->SBUF DMAs.
  - tap tiles are built by SBUF->SBUF DMAs straight from SBUF (no DRAM
    bounce).
  - every 8 steps the per-sample sum is folded out as a raw scale; the
    reciprocal runs on GPSIMD with 2 steps of slack (first two on DVE
    while GPSIMD is still busy with band products).  The final alpha row
    and the raw scales go to the host, which finishes with log()s.
  - same-engine (DVE->DVE) scheduler dependencies are demoted from
    semaphore-synced to program-order-only before wait assignment: engine
    execution is in-order and the DVE drains its pipe between ops, so the
    semaphore round-trip (~95ns/op on the serial chain) is pure overhead.
"""

import os

import numpy as np

import bass_rust as _bass_rust
import concourse.bacc as bacc
import concourse.bass as bass
import concourse.mybir as mybir
import concourse.tile as tile
from concourse.bass_utils import run_bass_kernel_spmd

T = 160
B = 64
C = 6625
S = 25
L = 2 * S + 1  # 51
NCORES = 8
BLOC = B // NCORES  # 8 samples per core
NORM_EVERY = 8
NSC = (T - 2) // NORM_EVERY  # 19 rescales at t=7,15,...,151
NG = 4  # gather blocks: partition p = b*16 + l//4, block j = l%4
LMASK_OFF = -100.0  # exp(score - 100) == 0 for all practical purposes

K1 = 48  # steps 1..K1-1 run unfused (their taps arrive first)
NPAIR = (T - K1) // 2  # 60 fused pairs: (40,41) ... (158,159)
NPH = 32  # pairs per precompute half (pair dim padded to NPH)

F32 = mybir.dt.float32
I32 = mybir.dt.int32
ALU = mybir.AluOpType
ACTF = mybir.ActivationFunctionType
AXIS = mybir.AxisListType

APPLY_DELAY = 2  # scale captured at t=8k-1 is divided out at t=8k+1


def _demote_same_engine_syncs(ordered):
    """Demote DVE->DVE sync dependencies to nosync (program order only).

    Engine execution is in-order and the DVE flushes its 8-slice pipe
    between ops, so same-engine hazards are already safe; the semaphore
    round-trip only adds ~95ns of propagation per op on the serial DP
    chain.  DMA copies complete asynchronously and keep their syncs, as
    does everything cross-engine.
    """
    info = {}
    for insts in ordered.values():
        for i in insts:
            info[i.name] = (i.engine, i.opcode)
    for insts in ordered.values():
        for i in insts:
            if i.engine != mybir.EngineType.DVE or i.opcode == "DMACopy":
                continue
            syncs = list(i.sync_dependency_names())
            demote = [
                d
                for d in syncs
                if info.get(d, (None, None))[0] == mybir.EngineType.DVE
                and info[d][1] != "DMACopy"
            ]
            if not demote:
                continue
            keep = [d for d in syncs if d not in set(demote)]
            cur_ns = list(i.nosync_dependency_names())
            i.take_sync_dependencies()
            i.take_nosync_dependencies()
            i.set_sync_dependencies(_bass_rust.InstructionNameOrderedSet(keep))
            i.set_nosync_dependencies(
                _bass_rust.InstructionNameOrderedSet(
                    list(dict.fromkeys(cur_ns + demote))
                )
            )


_ORIG_TCW = tile.TileClockWait


def _demoting_tcw(tc, ordered):
    _demote_same_engine_syncs(ordered)
    return _ORIG_TCW(tc, ordered)


def build_nc(loop_T: int = T) -> bass.Bass:
    if os.environ.get("CTC_NO_DEMOTE", "0") != "1":
        tile.TileClockWait = _demoting_tcw
    try:
        nc = bacc.Bacc("TRN2", target_bir_lowering=False)

        preds = nc.dram_tensor("preds", [BLOC * C, T], F32, kind="ExternalInput")
        # gidx columns: [main | s-1 | s-2]; bias columns:
        # [q-main | p(s-1) zero | q(s-1) | p(s-2) zero | q(s-2)]
        gidx_d = nc.dram_tensor("gidx", [128, 3 * NG], I32, kind="ExternalInput")
        lmask_d = nc.dram_tensor("lmask", [128, 5 * NG], F32, kind="ExternalInput")
        out_d = nc.dram_tensor("out2", [BLOC, L + NSC], F32, kind="ExternalOutput")
        debug = os.environ.get("CTC_DEBUG", "0") == "1"
        if debug:
            dbg3_d = nc.dram_tensor(
                "dbg3", [BLOC, 64 * 3 * K1], F32, kind="ExternalOutput"
            )
            dbg5_d = nc.dram_tensor(
                "dbg5", [BLOC, 64 * 5 * NPH], F32, kind="ExternalOutput"
            )
            dbgq_d = nc.dram_tensor("dbgq", [128, NG * T], F32, kind="ExternalOutput")

        W_G = NG * T  # per-partition row width of the gather-layout tiles
        W3 = 64 * 3 * K1
        W5H = 64 * 5 * NPH
        W_ND = NG * 5 * NPH
        W_I = NG * NPAIR  # row width of the product intermediates

        with tile.TileContext(nc) as tc:
            with (
                tc.tile_pool(name="big", bufs=1) as bigp,
                tc.tile_pool(name="small", bufs=1) as smallp,
                tc.tile_pool(name="tmp", bufs=2) as tmpp,
            ):
                G = bigp.tile([128, NG, T], F32, tag="G")
                Gp = bigp.tile([128, NG, T], F32, tag="Gp")
                Gq = bigp.tile([128, NG, T], F32, tag="Gq")
                G1 = bigp.tile([128, NG, T], F32, tag="G1")  # scores at s-1
                G2 = bigp.tile([128, NG, T], F32, tag="G2")  # scores at s-2
                Gp1 = bigp.tile([128, NG, T], F32, tag="Gp1")  # p[s-1]
                Gp2 = bigp.tile([128, NG, T], F32, tag="Gp2")  # p[s-2]
                Gq1 = bigp.tile([128, NG, T], F32, tag="Gq1")  # q[s-1]
                Gq2 = bigp.tile([128, NG, T], F32, tag="Gq2")  # q[s-2]
                # unfused taps for steps < K1 (state dim padded 51->64 so
                # (b, l) merges into legal partition-major DMA dims)
                PPQ3 = bigp.tile([BLOC, 64, 3, K1], F32, tag="PPQ3")
                # fused 5-band taps, two halves of NPH pairs each
                PPQ5 = [
                    bigp.tile([BLOC, 64, 5, NPH], F32, tag=f"PPQ5{h}", name=f"PPQ5{h}")
                    for h in range(2)
                ]
                ND = [
                    bigp.tile([128, NG, 5, NPH], F32, tag=f"ND{h}", name=f"ND{h}")
                    for h in range(2)
                ]
                AA = bigp.tile([128, NG, NPAIR], F32, tag="AA")
                BB = bigp.tile([128, NG, NPAIR], F32, tag="BB")
                U2 = bigp.tile([128, NG, NPAIR], F32, tag="U2")
                T2 = bigp.tile([128, NG, NPAIR], F32, tag="T2")
                T3 = bigp.tile([128, NG, NPAIR], F32, tag="T3")

                gidx = smallp.tile([128, NG], I32, tag="gidx")
                gidx1 = smallp.tile([128, NG], I32, tag="gidx1")
                gidx2 = smallp.tile([128, NG], I32, tag="gidx2")
                lmask = smallp.tile([128, 5 * NG], F32, tag="lmask")
                X = smallp.tile([BLOC, L + 4], F32, tag="X")
                Y = smallp.tile([BLOC, L + 4], F32, tag="Y")
                scales = smallp.tile([BLOC, NSC], F32, tag="scales")
                rcol = smallp.tile([BLOC, 2], F32, tag="rcol")
                ones = smallp.tile([BLOC, 1], F32, tag="ones")
                rtmp = smallp.tile([BLOC, 2], F32, tag="rtmp")
                dummy = smallp.tile([BLOC, 1], F32, tag="dummy")

                # Trigger the Exp act-table load immediately (1.3us; would
                # otherwise land on the first-chunk critical path).
                nc.vector.memset(dummy[:, :], 0.0)
                nc.scalar.activation(dummy[:, :], dummy[:, :], ACTF.Exp)
                nc.gpsimd.memset(ones[:, :], 1.0)
                # the last precompute half covers only NPAIR-NPH pairs; zero
                # the pad pair-columns its interleave DMA will still read
                pad0 = NPAIR - NPH
                nd1 = ND[1][:, :, :, :]
                nc.vector.memset(
                    bass.AP(
                        nd1.tensor,
                        nd1.offset + pad0,
                        [[W_ND, 128], [NPH, 5 * NG], [1, NPH - pad0]],
                    ),
                    0.0,
                )

                nc.sync.dma_start(out=gidx[:, :], in_=gidx_d[:, 0:NG])
                nc.sync.dma_start(out=gidx1[:, :], in_=gidx_d[:, NG : 2 * NG])
                nc.sync.dma_start(out=gidx2[:, :], in_=gidx_d[:, 2 * NG : 3 * NG])
                nc.sync.dma_start(out=lmask[:, :], in_=lmask_d[:, :])

                # Multi-row indirect gathers: main rows first (the DP-start
                # critical path), then the state-shifted row sets used only
                # by the band precompute.
                # one DMA per index column: multi-column offset APs are
                # broken in the HW DGE path (device-unrecoverable crash)
                for gt, gx in ((G, gidx), (G1, gidx1), (G2, gidx2)):
                    for j in range(NG):
                        nc.gpsimd.indirect_dma_start(
                            out=gt[:, j, :],
                            out_offset=None,
                            in_=preds[:, :],
                            in_offset=bass.IndirectOffsetOnAxis(
                                ap=gx[:, j : j + 1], axis=0
                            ),
                        )

                def exp_cols(c0, c1):
                    cs = slice(c0, c1)
                    nc.scalar.activation(Gp[:, :, cs], G[:, :, cs], ACTF.Exp)
                    for j in range(NG):
                        nc.scalar.activation(
                            Gq[:, j, cs],
                            G[:, j, cs],
                            ACTF.Exp,
                            bias=lmask[:, j : j + 1],
                        )

                # chunk 0 feeds the unfused region; build its (q,p,p) taps
                exp_cols(0, K1)
                gp_ap = Gp[:, :, :]
                gq_ap = Gq[:, :, :]
                p3_ap = PPQ3[:, :, :, :]
                in3 = [[W_G, 128], [T, NG], [1, K1]]
                out3 = [[W3, BLOC], [3 * K1, 64], [1, K1]]
                nc.scalar.dma_start(
                    out=bass.AP(p3_ap.tensor, p3_ap.offset + 0 * K1, out3),
                    in_=bass.AP(gq_ap.tensor, gq_ap.offset, in3),
                )
                nc.sync.dma_start(
                    out=bass.AP(p3_ap.tensor, p3_ap.offset + 1 * K1, out3),
                    in_=bass.AP(gp_ap.tensor, gp_ap.offset, in3),
                )
                nc.sync.dma_start(
                    out=bass.AP(p3_ap.tensor, p3_ap.offset + 2 * K1, out3),
                    in_=bass.AP(gp_ap.tensor, gp_ap.offset, in3),
                )

                # rest of the time axis, then state-shifted copies
                exp_cols(K1, T)

                # Shifted-row exps for the band precompute (t >= K1 only).
                # The l=0 (resp. l<2) boundary rows and the skip mask are
                # folded into the exp bias (-100 => exp ~ 0).
                cs = slice(K1, T)
                for j in range(NG):
                    nc.scalar.activation(
                        Gp1[:, j, cs], G1[:, j, cs], ACTF.Exp,
                        bias=lmask[:, NG + j : NG + j + 1],
                    )
                    nc.scalar.activation(
                        Gq1[:, j, cs], G1[:, j, cs], ACTF.Exp,
                        bias=lmask[:, 2 * NG + j : 2 * NG + j + 1],
                    )
                    nc.scalar.activation(
                        Gp2[:, j, cs], G2[:, j, cs], ACTF.Exp,
                        bias=lmask[:, 3 * NG + j : 3 * NG + j + 1],
                    )
                    nc.scalar.activation(
                        Gq2[:, j, cs], G2[:, j, cs], ACTF.Exp,
                        bias=lmask[:, 4 * NG + j : 4 * NG + j + 1],
                    )

                # 5-band products on GPSIMD.  For the pair (t, t+1) with
                # p1=p[t], q1=q[t], p2=p[t+1], q2=q[t+1] (t = K1+2n):
                #   N0 = p2*p1
                #   N1 = p2*(p1 + p1[s-1])
                #   N2 = p2*(q1 + p1[s-1]) + q2*p1[s-2]
                #   N3 = p2*q1[s-1] + q2*p1[s-2]
                #   N4 = q2*q1[s-2]
                def pview(tl, t0, n):  # [128, NG, n] view over t = t0, t0+2, ...
                    ap = tl[:, :, :]
                    return bass.AP(
                        ap.tensor, ap.offset + t0, [[W_G, 128], [T, NG], [2, n]]
                    )

                def iview(tl, n0, cnt):  # intermediate [128, NG, cnt] at n0
                    ap = tl[:, :, :]
                    return bass.AP(
                        ap.tensor, ap.offset + n0, [[W_I, 128], [NPAIR, NG], [1, cnt]]
                    )

                gp = nc.gpsimd
                gp.tensor_tensor(
                    out=AA[:, :, :], in0=pview(Gp, K1, NPAIR),
                    in1=pview(Gp1, K1, NPAIR), op=ALU.add,
                )
                gp.tensor_tensor(
                    out=BB[:, :, :], in0=pview(Gq, K1, NPAIR),
                    in1=pview(Gp1, K1, NPAIR), op=ALU.add,
                )
                gp.tensor_tensor(
                    out=U2[:, :, :], in0=pview(Gq, K1 + 1, NPAIR),
                    in1=pview(Gp2, K1, NPAIR), op=ALU.mult,
                )
                gp.tensor_tensor(
                    out=T2[:, :, :], in0=pview(Gp, K1 + 1, NPAIR),
                    in1=BB[:, :, :], op=ALU.mult,
                )
                gp.tensor_tensor(
                    out=T3[:, :, :], in0=pview(Gp, K1 + 1, NPAIR),
                    in1=pview(Gq1, K1, NPAIR), op=ALU.mult,
                )
                for h in range(2):
                    n0 = h * NPH
                    cnt = min(NPAIR, n0 + NPH) - n0
                    ndh = ND[h][:, :, :, :]

                    def nd_out(d):
                        # tap axis j reads alpha[s-4+j], so band d lands
                        # in tap slot 4-d
                        return bass.AP(
                            ndh.tensor,
                            ndh.offset + (4 - d) * NPH,
                            [[W_ND, 128], [5 * NPH, NG], [1, cnt]],
                        )

                    gp.tensor_tensor(
                        out=nd_out(0), in0=pview(Gp, K1 + 1 + 2 * n0, cnt),
                        in1=pview(Gp, K1 + 2 * n0, cnt), op=ALU.mult,
                    )
                    gp.tensor_tensor(
                        out=nd_out(1), in0=pview(Gp, K1 + 1 + 2 * n0, cnt),
                        in1=iview(AA, n0, cnt), op=ALU.mult,
                    )
                    gp.tensor_tensor(
                        out=nd_out(2), in0=iview(T2, n0, cnt),
                        in1=iview(U2, n0, cnt), op=ALU.add,
                    )
                    gp.tensor_tensor(
                        out=nd_out(3), in0=iview(T3, n0, cnt),
                        in1=iview(U2, n0, cnt), op=ALU.add,
                    )
                    gp.tensor_tensor(
                        out=nd_out(4), in0=pview(Gq, K1 + 1 + 2 * n0, cnt),
                        in1=pview(Gq2, K1 + 2 * n0, cnt), op=ALU.mult,
                    )

                    # interleave into the [b, l, d, n] tap tile
                    p5 = PPQ5[h][:, :, :, :]
                    nc.sync.dma_start(
                        out=bass.AP(
                            p5.tensor,
                            p5.offset,
                            [[W5H, BLOC], [5 * NPH, 64], [1, 5 * NPH]],
                        ),
                        in_=bass.AP(
                            ndh.tensor,
                            ndh.offset,
                            [[W_ND, 128], [5 * NPH, NG], [1, 5 * NPH]],
                        ),
                    )

                # alpha0: [p(0,0), p(0,1), 0, ...] in padded cols 4:6 of X
                nc.vector.memset(X[:, :], 0.0)
                nc.vector.memset(Y[:, :], 0.0)
                nc.vector.tensor_copy(X[:, 4:6], PPQ3[:, 0:2, 1, 0])

                state = {"cur": X, "nxt": Y}
                apply_step = {}  # unfused step t -> scale index
                apply_pair = {}  # fused pair n -> scale index

                def dp_op(taps, coef_view, is_norm, k, ak):
                    cur, nxt = state["cur"], state["nxt"]
                    xap = cur[:, :]
                    xxx = bass.AP(
                        xap.tensor,
                        xap.offset + (4 - (taps - 1)),
                        [xap.ap[0], [1, L], [1, taps]],
                    )
                    M = tmpp.tile([BLOC, L, taps], F32, tag=f"M{taps}")
                    if ak is not None or is_norm:
                        nc.vector.scalar_tensor_tensor(
                            out=M[:, :, :],
                            in0=xxx,
                            scalar=rcol[:, ak % 2 : ak % 2 + 1]
                            if ak is not None
                            else 1.0,
                            in1=coef_view,
                            op0=ALU.mult,
                            op1=ALU.mult,
                            accum_out=scales[:, k : k + 1] if is_norm else None,
                        )
                    else:
                        nc.vector.tensor_tensor(
                            out=M[:, :, :], in0=xxx, in1=coef_view, op=ALU.mult
                        )
                    nc.vector.tensor_reduce(
                        out=nxt[:, 4 : L + 4],
                        in_=M[:, :, :],
                        axis=AXIS.X,
                        op=ALU.add,
                    )
                    if is_norm:
                        if k < 4:
                            # gpsimd is still busy with band products this
                            # early; a DVE reciprocal costs only ~120ns
                            nc.vector.reciprocal(
                                out=rcol[:, k % 2 : k % 2 + 1],
                                in_=scales[:, k : k + 1],
                            )
                        else:
                            nc.gpsimd.tensor_copy(
                                rtmp[:, k % 2 : k % 2 + 1], scales[:, k : k + 1]
                            )
                            nc.gpsimd.normalize_recip(
                                out_ap=rcol[:, k % 2 : k % 2 + 1],
                                in_ap=ones[:, :],
                                denom_ap=rtmp[:, k % 2 : k % 2 + 1],
                            )
                    state["cur"], state["nxt"] = nxt, cur

                # unfused region: steps 1 .. K1-1
                for t in range(1, K1):
                    is_norm = t % NORM_EVERY == NORM_EVERY - 1
                    k = t // NORM_EVERY
                    ak = apply_step.pop(t, None)
                    dp_op(3, PPQ3[:, 0:L, :, t], is_norm, k, ak)
                    if is_norm:
                        if t + APPLY_DELAY < K1:
                            apply_step[t + APPLY_DELAY] = k
                        else:
                            apply_pair[(t + APPLY_DELAY - K1) // 2] = k

                # fused region: pairs n=0..NPAIR-1 covering (K1+2n, K1+2n+1)
                for n in range(NPAIR):
                    h, nn = divmod(n, NPH)
                    t_end = K1 + 2 * n + 1
                    is_norm = t_end % NORM_EVERY == NORM_EVERY - 1 and t_end < T - 1
                    k = t_end // NORM_EVERY
                    ak = apply_pair.pop(n, None)
                    dp_op(5, PPQ5[h][:, 0:L, :, nn], is_norm, k, ak)
                    if is_norm:
                        apply_pair[n + 1] = k

                if debug:
                    nc.sync.dma_start(
                        out=dbg3_d[:, :],
                        in_=bass.AP(p3_ap.tensor, p3_ap.offset, [[W3, BLOC], [1, W3]]),
                    )
                    p50 = PPQ5[0][:, :, :, :]
                    nc.sync.dma_start(
                        out=dbg5_d[:, :],
                        in_=bass.AP(p50.tensor, p50.offset, [[W5H, BLOC], [1, W5H]]),
                    )
                    nc.sync.dma_start(
                        out=dbgq_d[:, :],
                        in_=bass.AP(gq_ap.tensor, gq_ap.offset, [[W_G, 128], [1, W_G]]),
                    )

                # Ship raw final alpha and raw window sums; host does logs.
                cur = state["cur"]
                nc.sync.dma_start(out=out_d[:, 0:L], in_=cur[:, 4 : L + 4])
                nc.scalar.dma_start(out=out_d[:, L : L + NSC], in_=scales[:, :])

        nc.finalize()
    finally:
        tile.TileClockWait = _ORIG_TCW
    return nc


def host_prep(predictions, targets, target_lengths):
    """Host-side shard + index prep. Returns per-core input maps."""
    predictions = np.asarray(predictions, dtype=np.float32)
    targets = np.asarray(targets)
    target_lengths = np.asarray(target_lengths)

    ext = np.zeros((B, L), dtype=np.int64)
    ext[:, 1::2] = targets
    mask01 = np.zeros((B, L), dtype=np.float32)
    mask01[:, 3::2] = (targets[:, 1:] != targets[:, :-1]).astype(np.float32)

    in_maps = []
    for kk in range(NCORES):
        bsl = slice(kk * BLOC, (kk + 1) * BLOC)
        # [T, BLOC, C] -> [BLOC, C, T] contiguous -> flat [BLOC*C, T]
        pshard = np.ascontiguousarray(
            predictions[:, bsl, :].transpose(1, 2, 0)
        ).reshape(BLOC * C, T)
        gidx = np.zeros((128, 3 * NG), dtype=np.int32)
        lm = np.full((128, 5 * NG), LMASK_OFF, dtype=np.float32)
        for b in range(BLOC):
            gb = kk * BLOC + b
            for l in range(L):
                p, j = b * 16 + l // 4, l % 4
                gidx[p, j] = b * C + ext[gb, l]
                if mask01[gb, l] > 0:
                    lm[p, j] = 0.0
                if l >= 1:
                    gidx[p, NG + j] = b * C + ext[gb, l - 1]
                    lm[p, NG + j] = 0.0  # p(s-1) valid
                    if mask01[gb, l - 1] > 0:
                        lm[p, 2 * NG + j] = 0.0  # q(s-1)
                if l >= 2:
                    gidx[p, 2 * NG + j] = b * C + ext[gb, l - 2]
                    lm[p, 3 * NG + j] = 0.0  # p(s-2) valid
                    if mask01[gb, l - 2] > 0:
                        lm[p, 4 * NG + j] = 0.0  # q(s-2)
        in_maps.append({"preds": pshard, "gidx": gidx, "lmask": lm})
    return in_maps


_NC_CACHE = {}


def kernel(predictions, targets, target_lengths):
    if "nc" not in _NC_CACHE:
        _NC_CACHE["nc"] = build_nc()
    nc = _NC_CACHE["nc"]

    in_maps = host_prep(predictions, targets, target_lengths)
    res = run_bass_kernel_spmd(nc, in_maps, core_ids=list(range(NCORES)))
    return finish(res.results, target_lengths)


def finish(results, target_lengths):
    out = np.concatenate(
        [np.asarray(r["out2"]).reshape(BLOC, L + NSC) for r in results]
    )
    alpha, scales = out[:, :L].astype(np.float64), out[:, L:].astype(np.float64)
    lengths = np.asarray(target_lengths)
    idx = (2 * lengths).astype(np.int64)
    val = alpha[np.arange(B), idx] + alpha[np.arange(B), idx - 1]
    with np.errstate(divide="ignore", invalid="ignore"):
        nll = -(np.log(val) + np.log(scales).sum(axis=1))
    bad = ~np.isfinite(nll) | (nll >= 1e29)
    per = np.where(bad, 0.0, nll / lengths.astype(np.float64))
    return np.float32(per.mean())
